# revision 9
# baseline (speedup 1.0000x reference)
"""Trainium2 Bass kernel for nn_ConvTM2d (Tsetlin-machine conv layer).

Reference computation (jax, fp32):
    patches = unfold(x, 3x3, pad=1)                        # [B, 576, 3136]
    lits    = [patches, 1-patches]                         # [B*L, 1152]
    include = (sigmoid(w_include) > 0.5)                   # binary [256, 1152]
    clauses = exp(log(lits + 1e-6) @ include.T)            # [B*L, 256]
    logits  = clauses @ vote.T                             # [B*L, 128]
    out     = logits -> [B, 128, 56, 56]

Device formulation (fp8 DoubleRow, v4):
  - log-literal image Lt in fp8e4 [128ch, 2batch, 58, 64]: 64 ch of
    log(x+eps) + 64 ch of log(1-x+eps); row stride padded to 64 so
    vertically adjacent taps sit 64 B apart (DoubleRow pair stride must
    be 16B-aligned). x-space zero-padding becomes constant borders.
  - Lh = Lt shifted left by one column (DVE copy, slab-pipelined): lets
    the two remaining row-2 taps (2,0)+(2,1) form a DoubleRow pair with
    pair stride = (Lh base - Lt base), 16B-aligned by construction
    (asserted at build).
  - conv as FIVE matmuls per (rtile, clause-half): 4 fp8 DoubleRow
    (K=256 each: 3 vertical tap pairs + 1 horizontal pair) + 1 plain fp8
    (tap (2,2)).  A DR matmul takes the same N cycles as a plain one but
    contracts two taps -> 5*448 cycles for 9 taps vs 9*448 in bf16.
  - both clause halves accumulate into one PSUM tile [128, 1024]
    spanning 2 banks; one Exp per rtile reads [128, 2, 448] -> C fp8.
  - vote = ONE DoubleRow matmul per rtile (pair dim = clause halves),
    output into the just-freed first PSUM bank of the same tile; DVE
    casts logits to bf16, host upcasts to fp32.
  - batch dim lives in the free dims, so DMA/complement/Ln/conv all
    process both per-core batches per instruction.

Sharding: data-parallel over batch B=16 -> 2 batches per core x 8 cores.
Weight binarization + fp8 packing is done once on host (tiny).
"""

import numpy as np
import ml_dtypes

EPS = 1e-6
B_FULL = 16
N_CORES = 8
B_PER_CORE = B_FULL // N_CORES
C_IN = 64
H = W = 56
HP = 58          # padded image rows/cols (1px border)
HL, WL = 58, 64  # L image: rows x row-stride
N_CLAUSES = 256
N_CLASSES = 128
R_TILE = 8
N_RTILES = H // R_TILE           # 7
N_UNITS = B_PER_CORE * N_RTILES  # 14
FD = R_TILE * W                  # 448 free elems per unit
N_WARM_MM = 10
PS_LAG = 1   # vote for unit u emitted after conv of unit u+PS_LAG

LOG_EPS = float(np.log(EPS))
LOG_1PEPS = float(np.log(1.0 + EPS))

_CACHE = {}

# Test-only: scale log-literals by this factor so clause sums don't underflow
# exp() and the full dataflow can be validated numerically. None in production.
_DEBUG_SCALE = None

# conv matmul plan per clause half: (kind, i0, j, weight slot)
#   "dr": DoubleRow vertical pair taps (i0,j)+(i0+1,j), stride WL
#   "hp": DoubleRow horizontal pair taps (2,0)+(2,1) via Lh
#   "pl": plain fp8 matmul, tap (i0,j)
MM_PLAN = [
    ("dr", 0, 0, 0), ("dr", 0, 1, 2), ("dr", 0, 2, 4),
    ("hp", 2, 0, 6), ("pl", 2, 2, 8),
]
W_SLOTS = 9  # weight slots per clause half


def _build_program():
    import concourse.bacc as bacc
    import concourse.bass as bass
    import concourse.mybir as mybir
    import concourse.tile as tile
    from concourse._compat import get_trn_type

    f32 = mybir.dt.float32
    bf16 = mybir.dt.bfloat16
    fp8 = mybir.dt.float8e4
    Ln = mybir.ActivationFunctionType.Ln
    Exp = mybir.ActivationFunctionType.Exp
    DR = mybir.MatmulPerfMode.DoubleRow
    mult = mybir.AluOpType.mult
    add = mybir.AluOpType.add

    nc = bacc.Bacc(
        get_trn_type() or "TRN2",
        target_bir_lowering=False,
        debug=False,
        enable_asserts=False,
    )

    x_d = nc.dram_tensor("xs", [C_IN, B_PER_CORE, H, W], bf16,
                         kind="ExternalInput")
    w_d = nc.dram_tensor("wstat", [128, 2 * W_SLOTS, 128], fp8,
                         kind="ExternalInput")
    v_d = nc.dram_tensor("voteT", [128, 2, 128], fp8, kind="ExternalInput")
    y_d = nc.dram_tensor("y", [B_PER_CORE, N_CLASSES, H, W], bf16,
                         kind="ExternalOutput")

    with tile.TileContext(nc) as tc:
        with tc.tile_pool(name="wpool", bufs=1) as wpool, \
             tc.tile_pool(name="xpool", bufs=1) as xpool, \
             tc.tile_pool(name="lpool", bufs=1) as lpool, \
             tc.tile_pool(name="cpool", bufs=3) as cpool, \
             tc.tile_pool(name="opool", bufs=3) as opool:

            # --- constants / warmup ---
            wzb = wpool.tile([128, FD], bf16)
            nc.vector.memset(wzb[:], 0.0)
            eps_t = wpool.tile([128, 1], f32)
            nc.vector.memset(eps_t[:], EPS)

            # Pre-load the Ln ACT table off the critical path.
            dact = wpool.tile([128, 1], f32)
            ln_insts, exp_insts = [], []
            ln_insts.append(
                nc.scalar.activation(dact[:], eps_t[:], Ln, bias=eps_t[:]))

            # PE HAM warmup: dummy matmuls so the PE runs at 2.4GHz when
            # the first real matmul issues. Pool closes afterwards,
            # freeing its PSUM bank for cpsum.
            with tc.tile_pool(name="warmps", bufs=1, space="PSUM") as warmps:
                wps = warmps.tile([64, FD], f32)
                for _ in range(N_WARM_MM):
                    nc.tensor.matmul(wps[:], wzb[:, 0:64], wzb[:],
                                     start=True, stop=True)

            # L image with a per-row shifted twin: [ch, b, row, v, col]
            # v=0 normal, v=1 = same row shifted left one column. Row
            # stride RS=128, so vertical DR pairs use stride 128 and the
            # horizontal pair (2,0)+(2,1) uses stride 64 (v0 -> v1 of the
            # same row) -- both 16B-aligned, and every AP's address span
            # stays narrow so range-based dep tracking is precise.
            RS = 2 * WL           # 128: row stride
            BS = HL * RS          # batch plane stride
            Lt = lpool.tile([128, B_PER_CORE, HL, 2, WL], fp8, name="Lt")
            if _DEBUG_SCALE is not None:
                # debug-only: the scale pass reads the whole Lt tile incl.
                # the unused stride-padding columns
                nc.gpsimd.memset(Lt[:], 0.0)

            def conv_rhs(b, r0, mi):
                """moving-operand AP for MM_PLAN entry mi of rtile r0,
                batch b. DR kinds add the pair dim."""
                kind, i0, j, _ = MM_PLAN[mi]
                base = Lt[:, 0, 0, 0, 0]
                off = Lt.offset + b * BS + (r0 + i0) * RS + j
                dims = [list(Lt.ap[0])]
                if kind == "dr":
                    dims.append([RS, 2])
                elif kind == "hp":
                    dims.append([WL, 2])
                dims += [[RS, R_TILE], [1, W]]
                return bass.AP(base.tensor, off, dims)

            # --- x + weights in ---
            # x first on each queue (the Ln pipeline gates on it), weights
            # behind (first needed ~2us later by the first conv matmul).
            chunks = [(0, 16, nc.sync), (16, 32, nc.gpsimd),
                      (32, 48, nc.sync), (48, 56, nc.gpsimd)]
            xcks = []
            for ci, (lo, hi, eng) in enumerate(chunks):
                xc = xpool.tile([128, B_PER_CORE, 16, W], bf16,
                                name=f"xc{ci}", tag=f"xc{ci}")
                eng.dma_start(xc[0:64, :, 0:hi - lo, :], x_d[:, :, lo:hi, :])
                xcks.append(xc)
            wsb = wpool.tile([128, 2 * W_SLOTS, 128], fp8)
            vsb = wpool.tile([128, 2, 128], fp8)
            nc.sync.dma_start(wsb[:], w_d[:])
            nc.gpsimd.dma_start(vsb[:], v_d[:])

            # --- Lt border memsets (fp8 constants, v=0 plane) ---
            for half, val in ((slice(0, 64), LOG_EPS),
                              (slice(64, 128), LOG_1PEPS)):
                nc.gpsimd.memset(Lt[half, :, 0, 0, :], val)
                nc.gpsimd.memset(Lt[half, :, HP - 1, 0, :], val)
                nc.vector.memset(Lt[half, :, 1:HP - 1, 0, 0], val)
                nc.vector.memset(Lt[half, :, 1:HP - 1, 0, HP - 1], val)

            # --- log-literal production (8-row slabs, both batches) ---
            # Lh slab c (rows 2+8c..9+8c) = Lt[rows, col+1]; emitted right
            # after Ln slab c+1 so its implicit wait-on-Lt-writers lands
            # on the correct slab.
            lh_copies = []

            def emit_lh(c):
                r_lo = 2 + 8 * c
                cp = nc.vector.tensor_copy(
                    Lt[:, :, r_lo:r_lo + 8, 1, 0:W],
                    Lt[:, :, r_lo:r_lo + 8, 0, 1:1 + W])
                lh_copies.append(cp)

            for s in range(N_RTILES):
                r0 = s * R_TILE
                xc = xcks[r0 // 16]
                sl = slice(r0 % 16, r0 % 16 + R_TILE)
                nc.vector.tensor_scalar(
                    xc[64:128, :, sl, :], xc[0:64, :, sl, :],
                    -1.0, 1.0, mult, add)
                ln_insts.append(nc.scalar.activation(
                    Lt[:, :, 1 + r0:1 + r0 + R_TILE, 0, 1:1 + W],
                    xc[:, :, sl, :], Ln, bias=eps_t[:]))
                if _DEBUG_SCALE is None and s >= 1:
                    emit_lh(s - 1)
            if _DEBUG_SCALE is not None:
                nc.vector.tensor_scalar_mul(
                    Lt[:, :, :, 0, :], Lt[:, :, :, 0, :],
                    float(_DEBUG_SCALE))
                for c in range(N_RTILES):
                    emit_lh(c)
            else:
                emit_lh(N_RTILES - 1)

            # --- conv + exp + vote + out, software-pipelined per unit ---
            with tc.tile_pool(name="cpsum", bufs=4, space="PSUM") as cpsum:
                units = [(b, r) for b in range(B_PER_CORE)
                         for r in range(N_RTILES)]
                cps_tiles = [None] * N_UNITS
                C_tiles = [None] * N_UNITS

                def emit_conv(u):
                    b, r = units[u]
                    r0 = r * R_TILE
                    cps = cpsum.tile([128, 1024], f32, name="cps")
                    cps_tiles[u] = cps
                    for cc in range(2):
                        for mi, (kind, i0, j, slot) in enumerate(MM_PLAN):
                            widx = cc * W_SLOTS + slot
                            if kind == "pl":
                                lhsT = wsb[:, widx, :]
                            else:
                                lhsT = wsb[:, widx:widx + 2, :]
                            nc.tensor.matmul(
                                cps[:, cc * 512:cc * 512 + FD],
                                lhsT, conv_rhs(b, r0, mi),
                                start=(mi == 0),
                                stop=(mi == len(MM_PLAN) - 1),
                                perf_mode=(None if kind == "pl" else DR),
                            )
                    # exp over both halves in one ACT op -> fp8 C
                    C = cpool.tile([128, 2, FD], fp8, name="C")
                    C_tiles[u] = C
                    src = bass.AP(cps.tensor, cps.offset,
                                  [list(cps.ap[0]), [512, 2], [1, FD]])
                    exp_insts.append(nc.scalar.activation(C[:], src, Exp))

                def emit_vote(u):
                    b, r = units[u]
                    r0 = r * R_TILE
                    cps = cps_tiles[u]
                    nc.tensor.matmul(
                        cps[:, 0:FD], vsb[:, :, :], C_tiles[u][:, :, :],
                        start=True, stop=True, perf_mode=DR,
                    )
                    o = opool.tile([128, FD], bf16, name="o")
                    nc.vector.tensor_copy(o[:], cps[:, 0:FD])
                    nc.sync.dma_start(y_d[b, :, r0:r0 + R_TILE, :], o[:])

                for u in range(N_UNITS):
                    emit_conv(u)
                    if u >= PS_LAG:
                        emit_vote(u - PS_LAG)
                for u in range(N_UNITS - PS_LAG, N_UNITS):
                    emit_vote(u)

                # ACT phases contiguous: all Ln, then all Exp (one table
                # switch instead of thrashing).
                for e in exp_insts:
                    tile.add_dep_helper(e.ins, ln_insts[-1].ins, sync=False,
                                        reason="ACT table phase order")

    nc.compile()
    return nc


def _lit_index(k, i, j):
    """w_include column for literal (channel-partition k, tap (i,j))."""
    if k < 64:
        return k * 9 + i * 3 + j
    return 576 + (k - 64) * 9 + i * 3 + j


def _host_prep(w_include, vote):
    fp8 = ml_dtypes.float8_e4m3
    include = (w_include > 0.0).astype(np.float32)  # sigmoid(w)>0.5 <=> w>0

    # wstat [128, 18, 128]: 9 slots per clause half (MM_PLAN layout)
    wstat = np.zeros((128, 2 * W_SLOTS, 128), np.float32)
    ks = np.arange(128)
    for cc in range(2):
        for kind, i0, j, slot in MM_PLAN:
            if kind == "dr":
                taps = [(i0, j), (i0 + 1, j)]
            elif kind == "hp":
                taps = [(2, 0), (2, 1)]
            else:
                taps = [(i0, j)]
            for p, (i, jj) in enumerate(taps):
                cols = np.array([_lit_index(k, i, jj) for k in ks])
                wstat[:, cc * W_SLOTS + slot + p, :] = \
                    include[cc * 128:(cc + 1) * 128, cols].T

    # voteT [128, 2, 128]: [k, half, class] = vote[class, half*128 + k]
    voteT = np.empty((128, 2, 128), np.float32)
    for i in range(2):
        voteT[:, i, :] = vote[:, i * 128:(i + 1) * 128].T
    np.clip(voteT, -240.0, 240.0, out=voteT)

    return wstat.astype(fp8), voteT.astype(fp8)


def kernel(x, w_include, vote, _trace=False):
    from concourse import bass_utils

    x = np.asarray(x, dtype=np.float32)
    wstat, vT = _host_prep(np.asarray(w_include, dtype=np.float32),
                           np.asarray(vote, dtype=np.float32))

    if "nc" not in _CACHE:
        _CACHE["nc"] = _build_program()
    nc = _CACHE["nc"]

    in_maps = []
    for core in range(N_CORES):
        xs = x[core * B_PER_CORE:(core + 1) * B_PER_CORE]
        xs = np.ascontiguousarray(
            xs.transpose(1, 0, 2, 3)).astype(ml_dtypes.bfloat16)
        in_maps.append({"xs": xs, "wstat": wstat, "voteT": vT})

    res = bass_utils.run_bass_kernel_spmd(
        nc, in_maps, core_ids=list(range(N_CORES)), trace=_trace,
    )
    out = np.concatenate(
        [r["y"].astype(np.float32) for r in res.results], axis=0)
    if _trace:
        _CACHE["last_results"] = res
    return out
